# revision 35
# baseline (speedup 1.0000x reference)
"""AtomPosGNN Trainium2 kernel: 4-layer GraphConv (norm='both') over a dense
0/1 adjacency, SPMD across 8 NeuronCores, fp8 DoubleRow aggregation.

Sharding: nodes split 1024/core. Core m holds the full-height column block
A[:, m*1024:(m+1)*1024] (== row block transposed; A symmetric) as exact 0/1
fp8e4, resident in SBUF in GLOBAL rank order with the own-rank (diagonal)
block zeroed; the diagonal block is carried separately (al). This makes every
address in the kernel static: the own-rank contribution is computed early
from local z against al, and the gathered pass runs over all 8 rank blocks
of the AllGather output (the own-rank rows multiply the zeroed block and
contribute nothing). No per-core dynamic DMA offsets remain -- in the
previous design each dynamic fetch cost ~2.3us of sync-queue time reloading
its offset register from DRAM, which serialized the collectives.

fp8 scheme: adjacency entries are exactly representable in fp8e4, and the
aggregation is the only O(N^2) work, so it runs in fp8 with
perf_mode=DoubleRow. Hidden-layer features are softplus outputs (all
positive), so fp8 quantization error accumulates incoherently across the
~33 neighbors while the signal adds coherently. Weight matmuls are bf16
except layers 1-2 (fp8 DoubleRow; see below).

Degree norm r = 1/sqrt(max(deg,1)): src scale pre-applied to the features
before the AllGather; dst scale folds into the softplus activation's
per-partition scale operand. Exp and Ln are steered into the combined
natural_log_exp_and_others activation table (one table load total).

Per-layer structure:
- both dst-half passes (nj0: dst rows 0-511, nj1: 512-1023) accumulate
  concurrently in 4+4 PSUM banks; emission order = [local nj0+nj1 from al,
  gathered-A nj0+nj1, gathered-B nj0, evict nj0, epilogue rj0-3,
  gathered-B nj1, evict nj1, epilogue rj4-7], so all A-phase work runs
  before the B AllGather must land, and the epilogue reuses banks the
  evictions just freed (single 8-slot PSUM tag whose round-robin order
  matches exactly).
- epilogue per dst chunk rj: weight matmuls into one PSUM bank, EXP
  (r fold) + LN at full 512 width, z = r*softplus written fp8. PSUM
  eviction is split across the vector and scalar engines to halve the
  serial wall between aggregation end and the AllGather trigger. Layers
  1-2 run the weight matmul as 2 fp8 DoubleRow ops (feature-chunk pairs)
  instead of 4 bf16 ops -- their quantization error is averaged down by
  the following aggregation layers (end-to-end 1.16e-2 vs the 2e-2 gate);
  layer 3 feeds the output directly and stays bf16, as does layer 0
  (single feature chunk, nothing to pair).
- two AllGathers per layer: A = dst rows 0-511 (full H), B = rows 512-1023.
  Payloads are written permuted (row p*4+c for partition p) so each
  gathered output is re-fetched with per-rank static DMAs (2KB contiguous
  per partition) into SBUF, so aggregation matmuls pipeline rank-by-rank
  behind the fetches.
- adjacency loads as 8x 1MB partition-major DMAs on the scalar-engine HWDGE
  queue (degree colsum matmuls stream behind them), keeping the sync queue
  free so the layer-0 AllGather input fires the moment z0 is ready.
- no dummy warm-up collective: the CC entry barrier starts at NEFF init
  (~22us) and runs 22-39us regardless (run-to-run variance); the layer-0
  feature AllGather fires right after it. A ~25us block of throwaway
  matmuls on resident data keeps the HAM clock gate warm through the
  barrier window so layer 0 starts at 2.4GHz instead of 1.2GHz.
- layer-3 epilogue writes bf16 straight to the output (host casts to f32).
"""

import numpy as np
import ml_dtypes

N = 8192
NCORES = 8
L = N // NCORES          # 1024 local nodes per core
EMB = 125
POS = 3
IN = 128                 # EMB + POS
H = 512
RJ = L // 128            # 8 row chunks per core
NCH = 64                 # adjacency chunks per core (global order)

BF16 = ml_dtypes.bfloat16
F8 = ml_dtypes.float8_e4m3

_STATE = {}


def _build(use_bias):
    import concourse.bass as bass
    import concourse.mybir as mybir
    import concourse.tile as tile
    from concourse import bacc

    f32 = mybir.dt.float32
    bf16 = mybir.dt.bfloat16
    fp8 = mybir.dt.float8e4
    u32 = mybir.dt.uint32
    DR = mybir.MatmulPerfMode.DoubleRow

    nc = bacc.Bacc("TRN2", target_bir_lowering=False, debug=False,
                   num_devices=NCORES)

    from concourse.hw_specs import get_activation_tables
    EXP = mybir.ActivationFunctionType.Exp
    LN = mybir.ActivationFunctionType.Ln
    for name, funcs in get_activation_tables(nc.m.arch).items():
        if name != "natural_log_exp_and_others":
            funcs.discard(EXP)
            funcs.discard(LN)

    adj_dram = nc.declare_dram_parameter("adj", [128, NCH * L], fp8,
                                         isOutput=False)
    al_dram = nc.declare_dram_parameter("al", [128, RJ * L], fp8,
                                        isOutput=False)
    f0_dram = nc.declare_dram_parameter("f0", [128, RJ * IN], f32,
                                        isOutput=False)
    w0_dram = nc.declare_dram_parameter("w0", [128, H], bf16, isOutput=False)
    wx_dram = [nc.declare_dram_parameter(f"w{i}", [128, 4 * H], bf16,
                                         isOutput=False) for i in (1, 2, 3)]
    w8_dram = [nc.declare_dram_parameter(f"w{i}f8", [128, 4 * H], fp8,
                                         isOutput=False) for i in (1, 2)]
    b_dram = nc.declare_dram_parameter("b", [4, H], bf16, isOutput=False)
    oblk_dram = nc.declare_dram_parameter("oblk", [128, 32], fp8,
                                          isOutput=False)
    out_dram = nc.declare_dram_parameter("out", [L, H], bf16, isOutput=True)

    rg = [list(range(NCORES))]

    def allgather(ins_ap, outs_ap):
        nc.gpsimd.collective_compute(
            "AllGather", mybir.AluOpType.bypass, replica_groups=rg,
            ins=[ins_ap], outs=[outs_ap])

    with tile.TileContext(nc) as tc:
        with (
            tc.tile_pool(name="sb", bufs=1) as sb,
            tc.tile_pool(name="zp", bufs=1) as zp,
            tc.tile_pool(name="ep", bufs=2) as ep,
            tc.tile_pool(name="ps", bufs=8, space="PSUM") as ps,
            tc.tile_pool(name="dr", bufs=1, space="DRAM") as dr,
        ):
            # ---- small loads first, then the big adjacency streams ----
            ones_blk = sb.tile([128, 2, 16], fp8)
            nc.sync.dma_start(ones_blk[:].rearrange("p a b -> p (a b)"),
                              oblk_dram[:])
            al_sb = sb.tile([128, RJ, L], fp8)            # 8 KB/partition
            nc.sync.dma_start(al_sb[:].rearrange("p a b -> p (a b)"),
                              al_dram[:])
            f0s = sb.tile([128, RJ, IN], f32)
            nc.scalar.dma_start(f0s[:].rearrange("p a b -> p (a b)"),
                                f0_dram[:])
            ones16 = sb.tile([16, 1], bf16)
            ones_row_b = sb.tile([1, 128], bf16)
            ones_row_f = sb.tile([1, 128], f32)
            r_pp = sb.tile([128, RJ], f32)
            nc.vector.memset(ones16[:], 1.0)
            nc.vector.memset(ones_row_b[:], 1.0)
            nc.vector.memset(ones_row_f[:], 1.0)

            a_sb = sb.tile([128, NCH, L], fp8)            # 64 KB/partition
            for g in range(8):
                eng = nc.scalar if g < 4 else nc.sync
                eng.dma_start(
                    a_sb[:, 8 * g:8 * (g + 1), :].rearrange(
                        "p a b -> p (a b)"),
                    adj_dram[:, 8 * g * L:8 * (g + 1) * L])

            # weights (needed only ~70us in; queue after adjacency)
            w0_sb = sb.tile([128, 1, H], bf16)
            wx_sb = [sb.tile([128, 4, H], bf16, name=f"wx{i}")
                     for i in range(3)]
            b_sb = sb.tile([1, 4, H], bf16)
            w8_sb = [sb.tile([128, 4, H], fp8, name=f"w8_{i}")
                     for i in range(2)]
            for i in range(2):
                nc.scalar.dma_start(
                    w8_sb[i][:].rearrange("p a b -> p (a b)"),
                    w8_dram[i][:])
            nc.scalar.dma_start(w0_sb[:, 0, :], w0_dram[:])
            for i in range(3):
                nc.scalar.dma_start(
                    wx_sb[i][:].rearrange("p a b -> p (a b)"), wx_dram[i][:])
            for l in range(4):
                nc.scalar.dma_start(b_sb[:, l, :], b_dram[l:l + 1, :])

            # ---- degrees: colsum partials over al (arrives first) + the 8
            # adjacency groups, streaming behind the DMAs ----
            deg16_ps = [ps.tile([16, 512], f32, tag="acc", name=f"deg16ps{j}")
                        for j in range(2)]
            deg_srcs = [al_sb[:, 2 * c:2 * c + 2, :] for c in range(RJ // 2)]
            deg_srcs += [a_sb[:, 2 * c:2 * c + 2, :] for c in range(NCH // 2)]
            for kp, src in enumerate(deg_srcs):
                for j in range(2):
                    nc.tensor.matmul(deg16_ps[j][:], ones_blk[:],
                                     src[:, :, j * 512:(j + 1) * 512],
                                     start=(kp == 0),
                                     stop=(kp == len(deg_srcs) - 1),
                                     perf_mode=DR)
            d16 = sb.tile([16, 2, 512], bf16)
            for j in range(2):
                nc.vector.tensor_copy(d16[:, j, :], deg16_ps[j][:])
            rp_ps = ps.tile([128, RJ], f32, tag="acc", name="rpps")
            for c in range(RJ):
                njc = (c * 128) // 512
                cc = (c * 128) % 512
                nc.tensor.matmul(rp_ps[:, c:c + 1],
                                 d16[:, njc, cc:cc + 128],
                                 ones16[:, 0:1],
                                 start=True, stop=True)
            tpp = sb.tile([128, RJ], f32)
            tpp2 = sb.tile([128, RJ], f32)
            nc.vector.tensor_scalar_max(tpp[:], rp_ps[:], 1.0)
            nc.vector.reciprocal(tpp2[:], tpp[:])
            nc.scalar.sqrt(r_pp[:], tpp2[:])
            # preload the exp/ln activation table now (scalar engine idle,
            # inside the CC-barrier shadow); otherwise the first softplus
            # pays the 1.28us table load on layer 0's AllGather trigger chain
            actwarm = sb.tile([128, 1], f32)
            nc.scalar.activation(actwarm[:], tpp2[:, 0:1], EXP)

            # ---- PE warm-keeper: ~24us of throwaway matmuls on resident
            # data filling the CC-barrier window (PE idle >3.4us re-throttles
            # the HAM clock gate to 1.2GHz, which would make layer 0 run at
            # half speed when the first AllGather lands) ----
            warm_ps = ps.tile([16, 512], f32, tag="acc", name="warmps")
            NWARM = 136
            for wi in range(NWARM):
                kp = wi % 28
                nc.tensor.matmul(warm_ps[:], ones_blk[:],
                                 a_sb[:, 2 * kp:2 * kp + 2, 0:512],
                                 start=(wi == 0), stop=(wi == NWARM - 1),
                                 perf_mode=DR)

            if use_bias:
                ones_1 = sb.tile([128, 2, 16], fp8)
                nc.vector.memset(ones_1[:], 1.0)
                deg_ps = [ps.tile([1, 512], f32, tag="acc", name=f"degps{j}")
                          for j in range(2)]
                for kp, src in enumerate(deg_srcs):
                    for j in range(2):
                        nc.tensor.matmul(deg_ps[j][:], ones_1[:, :, 0:1],
                                         src[:, :, j * 512:(j + 1) * 512],
                                         start=(kp == 0),
                                         stop=(kp == len(deg_srcs) - 1),
                                         perf_mode=DR)
                t0 = sb.tile([1, L], f32)
                r_row = sb.tile([1, L], f32)
                for j in range(2):
                    nc.scalar.copy(t0[:, j * 512:(j + 1) * 512], deg_ps[j][:])
                nc.vector.tensor_scalar_max(r_row[:], t0[:], 1.0)
                nc.vector.reciprocal(t0[:], r_row[:])
                nc.scalar.sqrt(r_row[:], t0[:])
                r_bcast = sb.tile([128, L], f32)
                for j in range(2):
                    rb_ps = ps.tile([128, 512], f32, tag="acc",
                                    name=f"rbps{j}")
                    nc.tensor.matmul(rb_ps[:], ones_row_f[:],
                                     r_row[:, j * 512:(j + 1) * 512],
                                     start=True, stop=True)
                    nc.vector.tensor_copy(r_bcast[:, j * 512:(j + 1) * 512],
                                          rb_ps[:])

            # layer-0 stationary z0 = fp8(r * f0), AllGathered permuted
            # (row p*8+c) so the re-fetch is one static contiguous DMA
            zh0 = sb.tile([128, RJ, IN], fp8)
            ag_f0i = dr.tile([L, IN], fp8, tag="agf0i")
            ag_f0o = dr.tile([N, IN], fp8, tag="agf0o", addr_space="Shared")
            for rj in range(RJ):
                nc.vector.tensor_scalar_mul(zh0[:, rj, :], f0s[:, rj, :],
                                            r_pp[:, rj:rj + 1])
            nc.sync.dma_start(
                ag_f0i[:].rearrange("(p c) w -> p c w", p=128), zh0[:])
            allgather(ag_f0i[:], ag_f0o[:])

            # gathered layer-0 features: ONE static fetch of all 8 rank
            # blocks (own block multiplies the zeroed adjacency)
            zf0 = zp.tile([128, NCORES, RJ, IN], fp8, tag="zA", bufs=2,
                          name="zf0")
            f0v = ag_f0o[:].rearrange("(r p c) w -> p r c w", r=NCORES, p=128)
            for r in range(NCORES):
                nc.sync.dma_start(zf0[:, r, :, :], f0v[:, r, :, :])

            zst = [sb.tile([128, RJ, H], fp8, name=f"zst{i}", tag="zst",
                           bufs=2) for i in range(3)]
            hT = sb.tile([128, 4, L], bf16)
            hT8 = sb.tile([128, 4, L], fp8)
            o3 = sb.tile([128, RJ, H], bf16)

            zcur = {0: zf0, 1: zf0}   # per-phase gathered tiles

            # ---- layers ----
            # AllGather split per layer: A = dst rows 0-511 (rj0-3),
            # B = rows 512-1023 (rj4-7); finer splits lose because the CC
            # ring serializes ops and each carries ~5us fixed cost.
            SHIPS = {
                0: {3: (0, 0, 4, "zB"), 7: (1, 4, 4, "zB")},
                1: {3: (0, 0, 4, "zB"), 7: (1, 4, 4, "zB")},
                2: {3: (0, 0, 4, "zB"), 7: (1, 4, 4, "zB")},
            }
            for layer in range(4):
                ci_n = 1 if layer == 0 else 4
                w_l = w0_sb if layer == 0 else wx_sb[layer - 1]
                SHIP = SHIPS.get(layer, {})
                ag_io = {}
                if layer < 3:
                    for _rj, (ph, c0, cn, _t) in SHIP.items():
                        ai = dr.tile([cn * 128, H], fp8,
                                     tag=f"ag{layer}_{ph}i",
                                     name=f"ag{layer}_{ph}i")
                        ao = dr.tile([cn * 128 * NCORES, H], fp8,
                                     tag=f"ag{layer}_{ph}o",
                                     addr_space="Shared",
                                     name=f"ag{layer}_{ph}o")
                        ag_io[ph] = (ai, ao)

                zmap = [(ph, c0, cn) for (ph, c0, cn, _t)
                        in SHIPS.get(layer - 1, {}).values()]
                psA = [[ps.tile([128, 512], f32, tag="acc",
                                name=f"psA{layer}_{nj}_{ci}")
                        for ci in range(ci_n)] for nj in range(2)]
                npairs = RJ // 2 + 2 * NCORES * 2
                pcnt = [0, 0]

                def agg_pair(nj, k_adj, a_src, lhs_t):
                    a2 = a_src[:, k_adj:k_adj + 2,
                               nj * 512:(nj + 1) * 512]
                    for ci in range(ci_n):
                        nc.tensor.matmul(
                            psA[nj][ci][:],
                            lhs_t[:, :, ci * 128:(ci + 1) * 128],
                            a2,
                            start=(pcnt[nj] == 0),
                            stop=(pcnt[nj] == npairs - 1),
                            perf_mode=DR)
                    pcnt[nj] += 1

                def gat_lhs(r, c):
                    if layer == 0:
                        return zf0[:, r, c:c + 2, :]
                    for ph, c0, cn in zmap:
                        if c0 <= c < c0 + cn:
                            return zcur[ph][:, r, c - c0:c - c0 + 2, :]
                    raise AssertionError(c)

                # local diagonal block, both passes (available early --
                # this is the filler that buffers the A AllGather latency)
                for nj in range(2):
                    for cp in range(RJ // 2):
                        lhs = (zh0 if layer == 0 else
                               zst[layer - 1])[:, 2 * cp:2 * cp + 2, :]
                        agg_pair(nj, 2 * cp, al_sb, lhs)
                # gathered rows 0-511 (AllGather A), both passes
                for cp in range(2):
                    for nj in range(2):
                        for r in range(NCORES):
                            agg_pair(nj, r * RJ + 2 * cp, a_sb,
                                     gat_lhs(r, 2 * cp))
                # gathered rows 512-1023 (B): pass nj0, then evict+epilogue
                # of the first half while nj1's B runs
                znxt = {}
                for half in range(2):
                    nj = half
                    for r in range(NCORES):
                        for cp in range(2):
                            agg_pair(nj, r * RJ + 4 + 2 * cp, a_sb,
                                     gat_lhs(r, 4 + 2 * cp))
                    nsl = slice(nj * 512, (nj + 1) * 512)
                    f8mm = (layer in (1, 2)) and not use_bias
                    hTd = hT8 if f8mm else hT
                    for ci in range(ci_n):
                        if use_bias:
                            nc.vector.tensor_mul(hT[:, ci, nsl],
                                                 psA[nj][ci][:],
                                                 r_bcast[:, nsl])
                        elif ci % 2 == 0:
                            # eviction split across vector+scalar halves the
                            # serial wall gating the epilogue -> AG trigger
                            nc.vector.tensor_copy(hTd[:, ci, nsl],
                                                  psA[nj][ci][:])
                        else:
                            nc.scalar.copy(hTd[:, ci, nsl], psA[nj][ci][:])
                    for rj in range(4 * nj, 4 * nj + 4):
                        y_ps = ps.tile([128, H], f32, tag="acc",
                                       name=f"yps{layer}_{rj}")
                        if use_bias:
                            nc.tensor.matmul(y_ps[:], ones_row_b[:],
                                             b_sb[:, layer, :],
                                             start=True, stop=False)
                        if f8mm:
                            w8 = w8_sb[layer - 1]
                            rsl = slice(rj * 128, (rj + 1) * 128)
                            for ch in (0, 2):
                                nc.tensor.matmul(
                                    y_ps[:],
                                    hT8[:, ch:ch + 2, rsl],
                                    w8[:, ch:ch + 2, :],
                                    start=(ch == 0), stop=(ch == 2),
                                    perf_mode=DR)
                        else:
                            for ci in range(ci_n):
                                nc.tensor.matmul(
                                    y_ps[:],
                                    hT[:, ci, rj * 128:(rj + 1) * 128],
                                    w_l[:, ci, :],
                                    start=(ci == 0 and not use_bias),
                                    stop=(ci == ci_n - 1))
                        sc = 1.0 if use_bias else r_pp[:, rj:rj + 1]
                        ey = ep.tile([128, H], f32, tag="ey")
                        nc.scalar.activation(ey[:], y_ps[:], EXP, scale=sc)
                        if layer < 3:
                            sp = ep.tile([128, H], f32, tag="sp")
                            nc.scalar.activation(sp[:], ey[:], LN, bias=1.0)
                            nc.vector.tensor_scalar_mul(
                                zst[layer][:, rj, :], sp[:],
                                r_pp[:, rj:rj + 1])
                        else:
                            nc.scalar.activation(o3[:, rj, :], ey[:], LN,
                                                 bias=1.0)
                        if layer < 3 and rj in SHIP:
                            ph, c0, cn, ztag = SHIP[rj]
                            ai, ao = ag_io[ph]
                            nc.sync.dma_start(
                                ai[:].rearrange("(p c) w -> p c w", p=128),
                                zst[layer][:, c0:c0 + cn, :])
                            allgather(ai[:], ao[:])
                            znx = zp.tile([128, NCORES, cn, H], fp8,
                                          tag=ztag, bufs=2,
                                          name=f"z{layer}_{ph}")
                            ogv = ao[:].rearrange(
                                "(r p c) w -> p r c w", r=NCORES, p=128)
                            for r in range(NCORES):
                                nc.sync.dma_start(znx[:, r, :, :],
                                                  ogv[:, r, :, :])
                            znxt[ph] = znx
                        elif layer == 3 and rj % 4 == 3:
                            ov = out_dram.rearrange("(c p) w -> p c w",
                                                    p=128)
                            for rr in range(rj - 3, rj + 1):
                                nc.sync.dma_start(ov[:, rr:rr + 1, :],
                                                  o3[:, rr:rr + 1, :])
                if layer < 3:
                    zcur = znxt

    nc.compile()
    return nc


def _prep_shards(atom_pos, dist_adj, atom_emb, W0, b0, W1, b1, W2, b2, W3, b3):
    adj = np.asarray(dist_adj, dtype=np.float32).copy()
    np.fill_diagonal(adj, 0.0)          # reference removes self loops
    a_f8 = adj.astype(F8)               # entries are exactly 0/1
    feat0 = np.concatenate(
        [np.asarray(atom_emb, np.float32), np.asarray(atom_pos, np.float32)],
        axis=1)
    w0 = np.asarray(W0, np.float32).astype(BF16)
    wx = [np.asarray(w, np.float32).astype(BF16).reshape(4, 128, H)
          .transpose(1, 0, 2).reshape(128, 4 * H) for w in (W1, W2, W3)]
    b = np.stack([np.asarray(x, np.float32) for x in (b0, b1, b2, b3)]
                 ).astype(BF16)
    obk = np.zeros((128, 2, 16), dtype=F8)
    for m in range(8):
        obk[m * 16:(m + 1) * 16, :, m] = 1.0
    obk = obk.reshape(128, 32)

    def pmajor(x, nch):
        # [nch*128, w] -> [128, nch*w] partition-major repack
        w = x.shape[1]
        return np.ascontiguousarray(
            x.reshape(nch, 128, w).transpose(1, 0, 2).reshape(128, nch * w))

    in_maps = []
    for m in range(NCORES):
        sl = slice(m * L, (m + 1) * L)
        blk = a_f8[:, sl].copy()        # [N, L] column block, global order
        al = blk[sl].copy()             # local diagonal block
        blk[sl] = 0                     # own-rank rows zeroed in the global
        im = {"adj": pmajor(blk, NCH), "al": pmajor(al, RJ),
              "w1f8": wx[0].astype(F8), "w2f8": wx[1].astype(F8),
              "f0": pmajor(feat0[sl], RJ),
              "w0": w0, "w1": wx[0], "w2": wx[1], "w3": wx[2], "b": b,
              "oblk": obk}
        in_maps.append(im)
    return in_maps


def kernel(**inputs):
    from concourse.bass_utils import run_bass_kernel_spmd

    use_bias = any(
        np.any(np.asarray(inputs[f"b{i}"]) != 0) for i in range(4))
    key = ("nc", use_bias)
    if key not in _STATE:
        _STATE[key] = _build(use_bias)
    nc = _STATE[key]
    in_maps = _prep_shards(**inputs)
    res = run_bass_kernel_spmd(nc, in_maps, core_ids=list(range(NCORES)))
    out = np.concatenate([res.results[m]["out"] for m in range(NCORES)],
                         axis=0)
    return out.astype(np.float32)


# revision 36
# speedup vs baseline: 1.1546x; 1.1546x over previous
"""AtomPosGNN Trainium2 kernel: 4-layer GraphConv (norm='both') over a dense
0/1 adjacency, SPMD across 8 NeuronCores, fp8 DoubleRow aggregation.

Sharding: nodes split 1024/core. Core m holds the full-height column block
A[:, m*1024:(m+1)*1024] (== row block transposed; A symmetric) as exact 0/1
fp8e4, resident in SBUF in GLOBAL rank order with the own-rank (diagonal)
block zeroed; the diagonal block is carried separately (al). This makes every
address in the kernel static: the own-rank contribution is computed early
from local z against al, and the gathered pass runs over all 8 rank blocks
of the AllGather output (the own-rank rows multiply the zeroed block and
contribute nothing). No per-core dynamic DMA offsets remain -- in the
previous design each dynamic fetch cost ~2.3us of sync-queue time reloading
its offset register from DRAM, which serialized the collectives.

fp8 scheme: adjacency entries are exactly representable in fp8e4, and the
aggregation is the only O(N^2) work, so it runs in fp8 with
perf_mode=DoubleRow. Hidden-layer features are softplus outputs (all
positive), so fp8 quantization error accumulates incoherently across the
~33 neighbors while the signal adds coherently. Weight matmuls are bf16
except layers 1-2 (fp8 DoubleRow; see below).

Degree norm r = 1/sqrt(max(deg,1)): src scale pre-applied to the features
before the AllGather; dst scale folds into the softplus activation's
per-partition scale operand. Exp and Ln are steered into the combined
natural_log_exp_and_others activation table (one table load total).

Per-layer structure:
- both dst-half passes (nj0: dst rows 0-511, nj1: 512-1023) accumulate
  concurrently in 4+4 PSUM banks; emission order = [local nj0+nj1 from al,
  gathered-A nj0+nj1, gathered-B nj0, evict nj0, epilogue rj0-3,
  gathered-B nj1, evict nj1, epilogue rj4-7], so all A-phase work runs
  before the B AllGather must land, and the epilogue reuses banks the
  evictions just freed (single 8-slot PSUM tag whose round-robin order
  matches exactly).
- epilogue per dst chunk rj: weight matmuls into one PSUM bank, EXP
  (r fold) + LN at full 512 width, z = r*softplus written fp8. PSUM
  eviction is split across the vector and scalar engines to halve the
  serial wall between aggregation end and the AllGather trigger. Layers
  1-2 run the weight matmul as 2 fp8 DoubleRow ops (feature-chunk pairs)
  instead of 4 bf16 ops -- their quantization error is averaged down by
  the following aggregation layers (end-to-end 1.16e-2 vs the 2e-2 gate);
  layer 3 feeds the output directly and stays bf16, as does layer 0
  (single feature chunk, nothing to pair).
- two AllGathers per layer: A = dst rows 0-511 (full H), B = rows 512-1023.
  Payloads are written permuted (row p*4+c for partition p) so each
  gathered output is re-fetched with per-rank static DMAs (2KB contiguous
  per partition) into SBUF, so aggregation matmuls pipeline rank-by-rank
  behind the fetches.
- adjacency loads as 8x 1MB partition-major DMAs on the scalar-engine HWDGE
  queue (degree colsum matmuls stream behind them), keeping the sync queue
  free so the layer-0 AllGather input fires the moment z0 is ready.
- no dummy warm-up collective: the CC entry barrier starts at NEFF init
  (~22us) and runs 22-39us regardless (run-to-run variance); the layer-0
  feature AllGather fires right after it. A ~25us block of throwaway
  matmuls on resident data keeps the HAM clock gate warm through the
  barrier window so layer 0 starts at 2.4GHz instead of 1.2GHz.
- layer-3 epilogue writes bf16 straight to the output (host casts to f32).
"""

import numpy as np
import ml_dtypes

N = 8192
NCORES = 8
L = N // NCORES          # 1024 local nodes per core
EMB = 125
POS = 3
IN = 128                 # EMB + POS
H = 512
RJ = L // 128            # 8 row chunks per core
NCH = 64                 # adjacency chunks per core (global order)

BF16 = ml_dtypes.bfloat16
F8 = ml_dtypes.float8_e4m3

_STATE = {}


def _build(use_bias):
    import concourse.bass as bass
    import concourse.mybir as mybir
    import concourse.tile as tile
    from concourse import bacc

    f32 = mybir.dt.float32
    bf16 = mybir.dt.bfloat16
    fp8 = mybir.dt.float8e4
    u32 = mybir.dt.uint32
    DR = mybir.MatmulPerfMode.DoubleRow

    nc = bacc.Bacc("TRN2", target_bir_lowering=False, debug=False,
                   num_devices=NCORES)

    from concourse.hw_specs import get_activation_tables
    EXP = mybir.ActivationFunctionType.Exp
    LN = mybir.ActivationFunctionType.Ln
    for name, funcs in get_activation_tables(nc.m.arch).items():
        if name != "natural_log_exp_and_others":
            funcs.discard(EXP)
            funcs.discard(LN)

    adj_dram = nc.declare_dram_parameter("adj", [128, NCH * L], fp8,
                                         isOutput=False)
    al_dram = nc.declare_dram_parameter("al", [128, RJ * L], fp8,
                                        isOutput=False)
    f0_dram = nc.declare_dram_parameter("f0", [128, RJ * IN], f32,
                                        isOutput=False)
    w0_dram = nc.declare_dram_parameter("w0", [128, H], bf16, isOutput=False)
    wx_dram = [nc.declare_dram_parameter(f"w{i}", [128, 4 * H], bf16,
                                         isOutput=False) for i in (1, 2, 3)]
    w8_dram = [nc.declare_dram_parameter(f"w{i}f8", [128, 4 * H], fp8,
                                         isOutput=False) for i in (1, 2)]
    b_dram = nc.declare_dram_parameter("b", [4, H], bf16, isOutput=False)
    oblk_dram = nc.declare_dram_parameter("oblk", [128, 32], fp8,
                                          isOutput=False)
    out_dram = nc.declare_dram_parameter("out", [L, H], bf16, isOutput=True)

    rg = [list(range(NCORES))]

    def allgather(ins_ap, outs_ap):
        nc.gpsimd.collective_compute(
            "AllGather", mybir.AluOpType.bypass, replica_groups=rg,
            ins=[ins_ap], outs=[outs_ap])

    with tile.TileContext(nc) as tc:
        with (
            tc.tile_pool(name="sb", bufs=1) as sb,
            tc.tile_pool(name="zp", bufs=1) as zp,
            tc.tile_pool(name="ep", bufs=2) as ep,
            tc.tile_pool(name="ps", bufs=8, space="PSUM") as ps,
            tc.tile_pool(name="dr", bufs=1, space="DRAM") as dr,
        ):
            # ---- small loads first, then the big adjacency streams ----
            ones_blk = sb.tile([128, 2, 16], fp8)
            nc.sync.dma_start(ones_blk[:].rearrange("p a b -> p (a b)"),
                              oblk_dram[:])
            al_sb = sb.tile([128, RJ, L], fp8)            # 8 KB/partition
            nc.sync.dma_start(al_sb[:].rearrange("p a b -> p (a b)"),
                              al_dram[:])
            f0s = sb.tile([128, RJ, IN], f32)
            nc.scalar.dma_start(f0s[:].rearrange("p a b -> p (a b)"),
                                f0_dram[:])
            ones16 = sb.tile([16, 1], bf16)
            ones_row_b = sb.tile([1, 128], bf16)
            ones_row_f = sb.tile([1, 128], f32)
            r_pp = sb.tile([128, RJ], f32)
            nc.vector.memset(ones16[:], 1.0)
            nc.vector.memset(ones_row_b[:], 1.0)
            nc.vector.memset(ones_row_f[:], 1.0)

            a_sb = sb.tile([128, NCH, L], fp8)            # 64 KB/partition
            for g in range(8):
                eng = nc.scalar if g < 5 else nc.sync
                eng.dma_start(
                    a_sb[:, 8 * g:8 * (g + 1), :].rearrange(
                        "p a b -> p (a b)"),
                    adj_dram[:, 8 * g * L:8 * (g + 1) * L])

            # weights (needed only ~70us in; queue after adjacency)
            w0_sb = sb.tile([128, 1, H], bf16)
            wx_sb = [sb.tile([128, 4, H], bf16, name=f"wx{i}")
                     for i in range(3)]
            b_sb = sb.tile([1, 4, H], bf16)
            w8_sb = [sb.tile([128, 4, H], fp8, name=f"w8_{i}")
                     for i in range(2)]
            for i in range(2):
                nc.scalar.dma_start(
                    w8_sb[i][:].rearrange("p a b -> p (a b)"),
                    w8_dram[i][:])
            nc.scalar.dma_start(w0_sb[:, 0, :], w0_dram[:])
            for i in range(3):
                nc.scalar.dma_start(
                    wx_sb[i][:].rearrange("p a b -> p (a b)"), wx_dram[i][:])
            for l in range(4):
                nc.scalar.dma_start(b_sb[:, l, :], b_dram[l:l + 1, :])

            # ---- degrees: colsum partials over al (arrives first) + the 8
            # adjacency groups, streaming behind the DMAs ----
            deg16_ps = [ps.tile([16, 512], f32, tag="acc", name=f"deg16ps{j}")
                        for j in range(2)]
            deg_srcs = [al_sb[:, 2 * c:2 * c + 2, :] for c in range(RJ // 2)]
            deg_srcs += [a_sb[:, 2 * c:2 * c + 2, :] for c in range(NCH // 2)]
            for kp, src in enumerate(deg_srcs):
                for j in range(2):
                    nc.tensor.matmul(deg16_ps[j][:], ones_blk[:],
                                     src[:, :, j * 512:(j + 1) * 512],
                                     start=(kp == 0),
                                     stop=(kp == len(deg_srcs) - 1),
                                     perf_mode=DR)
            d16 = sb.tile([16, 2, 512], bf16)
            for j in range(2):
                nc.vector.tensor_copy(d16[:, j, :], deg16_ps[j][:])
            rp_ps = ps.tile([128, RJ], f32, tag="acc", name="rpps")
            for c in range(RJ):
                njc = (c * 128) // 512
                cc = (c * 128) % 512
                nc.tensor.matmul(rp_ps[:, c:c + 1],
                                 d16[:, njc, cc:cc + 128],
                                 ones16[:, 0:1],
                                 start=True, stop=True)
            tpp = sb.tile([128, RJ], f32)
            tpp2 = sb.tile([128, RJ], f32)
            nc.vector.tensor_scalar_max(tpp[:], rp_ps[:], 1.0)
            nc.vector.reciprocal(tpp2[:], tpp[:])
            nc.scalar.sqrt(r_pp[:], tpp2[:])
            # preload the exp/ln activation table now (scalar engine idle,
            # inside the CC-barrier shadow); otherwise the first softplus
            # pays the 1.28us table load on layer 0's AllGather trigger chain
            actwarm = sb.tile([128, 1], f32)
            nc.scalar.activation(actwarm[:], tpp2[:, 0:1], EXP)

            # ---- PE warm-keeper: ~24us of throwaway matmuls on resident
            # data filling the CC-barrier window (PE idle >3.4us re-throttles
            # the HAM clock gate to 1.2GHz, which would make layer 0 run at
            # half speed when the first AllGather lands) ----
            warm_ps = ps.tile([16, 512], f32, tag="acc", name="warmps")
            NWARM = 136
            for wi in range(NWARM):
                kp = wi % 28
                nc.tensor.matmul(warm_ps[:], ones_blk[:],
                                 a_sb[:, 2 * kp:2 * kp + 2, 0:512],
                                 start=(wi == 0), stop=(wi == NWARM - 1),
                                 perf_mode=DR)

            if use_bias:
                ones_1 = sb.tile([128, 2, 16], fp8)
                nc.vector.memset(ones_1[:], 1.0)
                deg_ps = [ps.tile([1, 512], f32, tag="acc", name=f"degps{j}")
                          for j in range(2)]
                for kp, src in enumerate(deg_srcs):
                    for j in range(2):
                        nc.tensor.matmul(deg_ps[j][:], ones_1[:, :, 0:1],
                                         src[:, :, j * 512:(j + 1) * 512],
                                         start=(kp == 0),
                                         stop=(kp == len(deg_srcs) - 1),
                                         perf_mode=DR)
                t0 = sb.tile([1, L], f32)
                r_row = sb.tile([1, L], f32)
                for j in range(2):
                    nc.scalar.copy(t0[:, j * 512:(j + 1) * 512], deg_ps[j][:])
                nc.vector.tensor_scalar_max(r_row[:], t0[:], 1.0)
                nc.vector.reciprocal(t0[:], r_row[:])
                nc.scalar.sqrt(r_row[:], t0[:])
                r_bcast = sb.tile([128, L], f32)
                for j in range(2):
                    rb_ps = ps.tile([128, 512], f32, tag="acc",
                                    name=f"rbps{j}")
                    nc.tensor.matmul(rb_ps[:], ones_row_f[:],
                                     r_row[:, j * 512:(j + 1) * 512],
                                     start=True, stop=True)
                    nc.vector.tensor_copy(r_bcast[:, j * 512:(j + 1) * 512],
                                          rb_ps[:])

            # layer-0 stationary z0 = fp8(r * f0), AllGathered permuted
            # (row p*8+c) so the re-fetch is one static contiguous DMA
            zh0 = sb.tile([128, RJ, IN], fp8)
            ag_f0i = dr.tile([L, IN], fp8, tag="agf0i")
            ag_f0o = dr.tile([N, IN], fp8, tag="agf0o", addr_space="Shared")
            for rj in range(RJ):
                nc.vector.tensor_scalar_mul(zh0[:, rj, :], f0s[:, rj, :],
                                            r_pp[:, rj:rj + 1])
            nc.sync.dma_start(
                ag_f0i[:].rearrange("(p c) w -> p c w", p=128), zh0[:])
            allgather(ag_f0i[:], ag_f0o[:])

            # gathered layer-0 features: ONE static fetch of all 8 rank
            # blocks (own block multiplies the zeroed adjacency)
            zf0 = zp.tile([128, NCORES, RJ, IN], fp8, tag="zA", bufs=2,
                          name="zf0")
            f0v = ag_f0o[:].rearrange("(r p c) w -> p r c w", r=NCORES, p=128)
            for r in range(NCORES):
                nc.sync.dma_start(zf0[:, r, :, :], f0v[:, r, :, :])

            zst = [sb.tile([128, RJ, H], fp8, name=f"zst{i}", tag="zst",
                           bufs=2) for i in range(3)]
            hT = sb.tile([128, 4, L], bf16)
            hT8 = sb.tile([128, 4, L], fp8)
            o3 = sb.tile([128, RJ, H], bf16)

            zcur = {0: zf0, 1: zf0}   # per-phase gathered tiles

            # ---- layers ----
            # AllGather split per layer: A = dst rows 0-511 (rj0-3),
            # B = rows 512-1023 (rj4-7); finer splits lose because the CC
            # ring serializes ops and each carries ~5us fixed cost.
            SHIPS = {
                0: {3: (0, 0, 4, "zB"), 7: (1, 4, 4, "zB")},
                1: {3: (0, 0, 4, "zB"), 7: (1, 4, 4, "zB")},
                2: {3: (0, 0, 4, "zB"), 7: (1, 4, 4, "zB")},
            }
            for layer in range(4):
                ci_n = 1 if layer == 0 else 4
                w_l = w0_sb if layer == 0 else wx_sb[layer - 1]
                SHIP = SHIPS.get(layer, {})
                ag_io = {}
                if layer < 3:
                    for _rj, (ph, c0, cn, _t) in SHIP.items():
                        ai = dr.tile([cn * 128, H], fp8,
                                     tag=f"ag{layer}_{ph}i",
                                     name=f"ag{layer}_{ph}i")
                        ao = dr.tile([cn * 128 * NCORES, H], fp8,
                                     tag=f"ag{layer}_{ph}o",
                                     addr_space="Shared",
                                     name=f"ag{layer}_{ph}o")
                        ag_io[ph] = (ai, ao)

                zmap = [(ph, c0, cn) for (ph, c0, cn, _t)
                        in SHIPS.get(layer - 1, {}).values()]
                psA = [[ps.tile([128, 512], f32, tag="acc",
                                name=f"psA{layer}_{nj}_{ci}")
                        for ci in range(ci_n)] for nj in range(2)]
                npairs = RJ // 2 + 2 * NCORES * 2
                pcnt = [0, 0]

                def agg_pair(nj, k_adj, a_src, lhs_t):
                    a2 = a_src[:, k_adj:k_adj + 2,
                               nj * 512:(nj + 1) * 512]
                    for ci in range(ci_n):
                        nc.tensor.matmul(
                            psA[nj][ci][:],
                            lhs_t[:, :, ci * 128:(ci + 1) * 128],
                            a2,
                            start=(pcnt[nj] == 0),
                            stop=(pcnt[nj] == npairs - 1),
                            perf_mode=DR)
                    pcnt[nj] += 1

                def gat_lhs(r, c):
                    if layer == 0:
                        return zf0[:, r, c:c + 2, :]
                    for ph, c0, cn in zmap:
                        if c0 <= c < c0 + cn:
                            return zcur[ph][:, r, c - c0:c - c0 + 2, :]
                    raise AssertionError(c)

                # local diagonal block, both passes (available early --
                # this is the filler that buffers the A AllGather latency)
                for nj in range(2):
                    for cp in range(RJ // 2):
                        lhs = (zh0 if layer == 0 else
                               zst[layer - 1])[:, 2 * cp:2 * cp + 2, :]
                        agg_pair(nj, 2 * cp, al_sb, lhs)
                # gathered rows 0-511 (AllGather A), both passes
                for cp in range(2):
                    for nj in range(2):
                        for r in range(NCORES):
                            agg_pair(nj, r * RJ + 2 * cp, a_sb,
                                     gat_lhs(r, 2 * cp))
                # gathered rows 512-1023 (B): pass nj0, then evict+epilogue
                # of the first half while nj1's B runs
                znxt = {}
                for half in range(2):
                    nj = half
                    for r in range(NCORES):
                        for cp in range(2):
                            agg_pair(nj, r * RJ + 4 + 2 * cp, a_sb,
                                     gat_lhs(r, 4 + 2 * cp))
                    nsl = slice(nj * 512, (nj + 1) * 512)
                    f8mm = (layer in (1, 2)) and not use_bias
                    hTd = hT8 if f8mm else hT
                    for ci in range(ci_n):
                        if use_bias:
                            nc.vector.tensor_mul(hT[:, ci, nsl],
                                                 psA[nj][ci][:],
                                                 r_bcast[:, nsl])
                        elif ci % 2 == 0:
                            # eviction split across vector+scalar halves the
                            # serial wall gating the epilogue -> AG trigger
                            nc.vector.tensor_copy(hTd[:, ci, nsl],
                                                  psA[nj][ci][:])
                        else:
                            nc.scalar.copy(hTd[:, ci, nsl], psA[nj][ci][:])
                    for rj in range(4 * nj, 4 * nj + 4):
                        y_ps = ps.tile([128, H], f32, tag="acc",
                                       name=f"yps{layer}_{rj}")
                        if use_bias:
                            nc.tensor.matmul(y_ps[:], ones_row_b[:],
                                             b_sb[:, layer, :],
                                             start=True, stop=False)
                        if f8mm:
                            w8 = w8_sb[layer - 1]
                            rsl = slice(rj * 128, (rj + 1) * 128)
                            for ch in (0, 2):
                                nc.tensor.matmul(
                                    y_ps[:],
                                    hT8[:, ch:ch + 2, rsl],
                                    w8[:, ch:ch + 2, :],
                                    start=(ch == 0), stop=(ch == 2),
                                    perf_mode=DR)
                        else:
                            for ci in range(ci_n):
                                nc.tensor.matmul(
                                    y_ps[:],
                                    hT[:, ci, rj * 128:(rj + 1) * 128],
                                    w_l[:, ci, :],
                                    start=(ci == 0 and not use_bias),
                                    stop=(ci == ci_n - 1))
                        sc = 1.0 if use_bias else r_pp[:, rj:rj + 1]
                        ey = ep.tile([128, H], f32, tag="ey")
                        nc.scalar.activation(ey[:], y_ps[:], EXP, scale=sc)
                        if layer < 3:
                            sp = ep.tile([128, H], f32, tag="sp")
                            nc.scalar.activation(sp[:], ey[:], LN, bias=1.0)
                            nc.vector.tensor_scalar_mul(
                                zst[layer][:, rj, :], sp[:],
                                r_pp[:, rj:rj + 1])
                        else:
                            nc.scalar.activation(o3[:, rj, :], ey[:], LN,
                                                 bias=1.0)
                        if layer < 3 and rj in SHIP:
                            ph, c0, cn, ztag = SHIP[rj]
                            ai, ao = ag_io[ph]
                            nc.sync.dma_start(
                                ai[:].rearrange("(p c) w -> p c w", p=128),
                                zst[layer][:, c0:c0 + cn, :])
                            allgather(ai[:], ao[:])
                            znx = zp.tile([128, NCORES, cn, H], fp8,
                                          tag=ztag, bufs=2,
                                          name=f"z{layer}_{ph}")
                            ogv = ao[:].rearrange(
                                "(r p c) w -> p r c w", r=NCORES, p=128)
                            for r in range(NCORES):
                                nc.sync.dma_start(znx[:, r, :, :],
                                                  ogv[:, r, :, :])
                            znxt[ph] = znx
                        elif layer == 3 and rj % 4 == 3:
                            ov = out_dram.rearrange("(c p) w -> p c w",
                                                    p=128)
                            for rr in range(rj - 3, rj + 1):
                                nc.sync.dma_start(ov[:, rr:rr + 1, :],
                                                  o3[:, rr:rr + 1, :])
                if layer < 3:
                    zcur = znxt

    nc.compile()
    return nc


def _prep_shards(atom_pos, dist_adj, atom_emb, W0, b0, W1, b1, W2, b2, W3, b3):
    adj = np.asarray(dist_adj, dtype=np.float32).copy()
    np.fill_diagonal(adj, 0.0)          # reference removes self loops
    a_f8 = adj.astype(F8)               # entries are exactly 0/1
    feat0 = np.concatenate(
        [np.asarray(atom_emb, np.float32), np.asarray(atom_pos, np.float32)],
        axis=1)
    w0 = np.asarray(W0, np.float32).astype(BF16)
    wx = [np.asarray(w, np.float32).astype(BF16).reshape(4, 128, H)
          .transpose(1, 0, 2).reshape(128, 4 * H) for w in (W1, W2, W3)]
    b = np.stack([np.asarray(x, np.float32) for x in (b0, b1, b2, b3)]
                 ).astype(BF16)
    obk = np.zeros((128, 2, 16), dtype=F8)
    for m in range(8):
        obk[m * 16:(m + 1) * 16, :, m] = 1.0
    obk = obk.reshape(128, 32)

    def pmajor(x, nch):
        # [nch*128, w] -> [128, nch*w] partition-major repack
        w = x.shape[1]
        return np.ascontiguousarray(
            x.reshape(nch, 128, w).transpose(1, 0, 2).reshape(128, nch * w))

    in_maps = []
    for m in range(NCORES):
        sl = slice(m * L, (m + 1) * L)
        blk = a_f8[:, sl].copy()        # [N, L] column block, global order
        al = blk[sl].copy()             # local diagonal block
        blk[sl] = 0                     # own-rank rows zeroed in the global
        im = {"adj": pmajor(blk, NCH), "al": pmajor(al, RJ),
              "w1f8": wx[0].astype(F8), "w2f8": wx[1].astype(F8),
              "f0": pmajor(feat0[sl], RJ),
              "w0": w0, "w1": wx[0], "w2": wx[1], "w3": wx[2], "b": b,
              "oblk": obk}
        in_maps.append(im)
    return in_maps


def kernel(**inputs):
    from concourse.bass_utils import run_bass_kernel_spmd

    use_bias = any(
        np.any(np.asarray(inputs[f"b{i}"]) != 0) for i in range(4))
    key = ("nc", use_bias)
    if key not in _STATE:
        _STATE[key] = _build(use_bias)
    nc = _STATE[key]
    in_maps = _prep_shards(**inputs)
    res = run_bass_kernel_spmd(nc, in_maps, core_ids=list(range(NCORES)))
    out = np.concatenate([res.results[m]["out"] for m in range(NCORES)],
                         axis=0)
    return out.astype(np.float32)
